# revision 11
# baseline (speedup 1.0000x reference)
"""DGCNN layer (dynamic kNN graph + edge MLP) for 8 Trainium2 cores.

Screen redesign vs the previous version: the kNN screen packs
(quantized score, in-window index) into a single fp32 PSUM value so the
DVE only makes ONE pass (max8) over the score matrix instead of two
(max8 + find_index8), and the score matmul runs in fp16 (1 cyc/col)
instead of fp32 (4 cyc/col).

Packing mechanism (hardware-verified): the PE accumulates one matmul's
K-rows in high precision and applies a single fp32 round on output, and
PSUM read-modify-write accumulation between chained matmuls rounds to
fp32 storage. Per 512-wide window, on one PSUM bank:
  m1 (K=70, fp16): 64 rows a*x_i * a*x_j (a^2 = 2C), 2 rows -C*(sq_j-80)
     split hi/lo, 4 rows summing to +M (M = 1.5*2^32). Output RNE at
     magnitude M quantizes: PSUM = M + 512*round(y/512), y = C*(score+80).
  m2 (K=4): -M. RMW cancels exactly: PSUM = Y.
  m3 (K=1): +iota_j. Y is a multiple of 512, j < 512: PSUM = Y + j exact.
max8 then yields top-8 packed values; j = P mod 512 recovered with exact
fp32 integer ops on the narrow coarse array. Quantization step 512/C =
1/120 in score units flips only near-ties (sim: rel err ~5e-3).

Edge MLP: layer 1 decomposed as relu(p_i + q_j + b1); the per-edge work
is a GPSIMD gather of q columns plus GPSIMD add/reduce (keeping DVE free
for the screen). Layer 2 is an fp16 matmul; relu/bias/mean-scale fused
into ACT evacuation. Output is produced transposed [C, rows]; the host
transposes back.
"""

import os
import sys

import numpy as np

N, D, C, K = 16384, 64, 128, 16
NCORES = 8
RPC = N // NCORES          # rows per core
BLK = 128                  # target rows per screen block
WIN = 512                  # screen window (one PSUM bank of fp32)
CHUNK = 512                # edges per MLP chunk
KA = D + 2 + 4             # m1 contraction rows (x + sq hi/lo + magic)

CSCALE = 61440.0           # score scale; quantum = 512/CSCALE
SHIFT = 80.0               # recenters scores so |y| stays under 2^23
MAGW = 32768.0             # magic piece = MAGW * MAGR = 3*2^29; 4 pieces = 1.5*2^32
MAGR = 49152.0
MAGF = float(1.5 * 2.0**23)  # fp32 integer-rounding constant for DVE
NEG = -3.0e38
MARK = float(1 << 20)

_REPO = "/opt/trn_rl_repo"


def _ensure_path():
    if _REPO not in sys.path:
        sys.path.insert(0, _REPO)


def build_program(n=N, d=D, c=C, k=K, rpc=RPC):
    _ensure_path()
    import concourse.mybir as mybir
    from concourse import tile
    from concourse.bacc import Bacc

    f32 = mybir.dt.float32
    f16 = mybir.dt.float16
    i16 = mybir.dt.int16

    nblk = rpc // BLK                    # 16
    nwin = n // WIN                      # 32
    ncoarse = nwin * 8                   # 256 coarse slots per row
    nchunk = (rpc * k) // CHUNK          # 64 MLP chunks
    rows_per_chunk = CHUNK // k          # 32
    jall_cols = max(128, ((nblk * k + 127) // 128) * 128)  # 256

    nc = Bacc()

    xaug_d = nc.declare_dram_parameter("xaug", [KA, n], f16, isOutput=False)
    wloc_d = nc.declare_dram_parameter("wloc", [KA, rpc], f16, isOutput=False)
    unmw_d = nc.declare_dram_parameter("unmw", [4, 128], f16, isOutput=False)
    unmr_d = nc.declare_dram_parameter("unmr", [4, WIN], f16, isOutput=False)
    iow_d = nc.declare_dram_parameter("iow", [1, 128], f16, isOutput=False)
    ior_d = nc.declare_dram_parameter("ior", [1, WIN], f16, isOutput=False)
    w1b_d = nc.declare_dram_parameter("w1b", [d, c], f16, isOutput=False)
    w1d_d = nc.declare_dram_parameter("w1d", [d, c], f16, isOutput=False)
    w2_d = nc.declare_dram_parameter("w2", [c, c], f16, isOutput=False)
    b1_d = nc.declare_dram_parameter("b1c", [c, 1], f32, isOutput=False)
    b2s_d = nc.declare_dram_parameter("b2s", [c, 1], f32, isOutput=False)
    wbase_d = nc.declare_dram_parameter("wbase", [128, ncoarse], f32, isOutput=False)
    out_d = nc.declare_dram_parameter("outT", [c, rpc], f32, isOutput=True)

    with tile.TileContext(nc) as tc:
        with (
            tc.tile_pool(name="const", bufs=1) as cpool,
            tc.tile_pool(name="screen", bufs=2) as spool,
            tc.tile_pool(name="small", bufs=2) as mpool,
            tc.tile_pool(name="mlp", bufs=3) as dpool,
            tc.tile_pool(name="psA", bufs=4, space="PSUM") as ppA,
            tc.tile_pool(name="psB", bufs=2, space="PSUM") as ppB,
        ):
            # ---- persistent tiles ----
            xaug = cpool.tile([KA, n], f16, tag="xaug")
            wloc = cpool.tile([KA, rpc], f16, tag="wloc")
            unmw = cpool.tile([4, 128], f16, tag="unmw")
            unmr = cpool.tile([4, WIN], f16, tag="unmr")
            iow = cpool.tile([1, 128], f16, tag="iow")
            ior = cpool.tile([1, WIN], f16, tag="ior")
            w1b = cpool.tile([d, c], f16, tag="w1b")
            w1d = cpool.tile([d, c], f16, tag="w1d")
            w2 = cpool.tile([c, c], f16, tag="w2")
            b1 = cpool.tile([c, 1], f32, tag="b1")
            b2s = cpool.tile([c, 1], f32, tag="b2s")
            wbase = cpool.tile([128, ncoarse], f32, tag="wbase")
            qT = cpool.tile([c, n], f32, tag="qT")
            pT = cpool.tile([c, rpc], f32, tag="pT")
            outT = cpool.tile([c, rpc], f32, tag="outT")
            jall = cpool.tile([128, jall_cols], i16, tag="jall")
            jT = cpool.tile([128, jall_cols], i16, tag="jT")
            jrep = [
                cpool.tile([128, BLK], i16, tag=f"jrep{b}", name=f"jrep{b}")
                for b in range(nblk)
            ]

            for t, dd in [(xaug, xaug_d), (wloc, wloc_d), (unmw, unmw_d),
                          (unmr, unmr_d), (iow, iow_d), (ior, ior_d),
                          (w1b, w1b_d), (w1d, w1d_d), (w2, w2_d),
                          (b1, b1_d), (b2s, b2s_d), (wbase, wbase_d)]:
                nc.sync.dma_start(t[:, :], dd[:, :])

            # ---- phase A: q = x.W1b (all nodes), p = x.(W1a-W1b) (local) ----
            # xaug rows 0:d hold a*x^T, so W1b/a etc. are pre-divided on host.
            for t in range(n // WIN):
                qp = ppB.tile([c, WIN], f32, tag="qp")
                nc.tensor.matmul(qp[:, :], w1b[:, :], xaug[0:d, t * WIN:(t + 1) * WIN])
                nc.scalar.activation(
                    qT[:, t * WIN:(t + 1) * WIN], qp[:, :],
                    mybir.ActivationFunctionType.Copy,
                )
            for b in range(nblk):
                pp = ppB.tile([c, BLK], f32, tag="qp")
                nc.tensor.matmul(pp[:, :], w1d[:, :], wloc[0:d, b * BLK:(b + 1) * BLK])
                nc.scalar.activation(
                    pT[:, b * BLK:(b + 1) * BLK], pp[:, :],
                    mybir.ActivationFunctionType.Copy,
                )

            def screen_block(b):
                """Packed screen for rows [b*BLK, (b+1)*BLK): writes jall."""
                cP = spool.tile([128, ncoarse], f32, tag="cP")
                for w in range(nwin):
                    ps = ppA.tile([128, WIN], f32, tag="scr")
                    nc.tensor.matmul(
                        ps[:, :],
                        wloc[:, b * BLK:(b + 1) * BLK],
                        xaug[:, w * WIN:(w + 1) * WIN],
                        start=True, stop=False,
                    )
                    nc.tensor.matmul(ps[:, :], unmw[:, :], unmr[:, :],
                                     start=False, stop=False)
                    nc.tensor.matmul(ps[:, :], iow[:, :], ior[:, :],
                                     start=False, stop=True)
                    nc.vector.max(cP[:, 8 * w:8 * w + 8], ps[:, :])

                # extraction: gj = wbase + (P mod 512), exact fp32 int ops
                t1 = mpool.tile([128, ncoarse], f32, tag="t1")
                t2 = mpool.tile([128, ncoarse], f32, tag="t2")
                dfr = mpool.tile([128, ncoarse], f32, tag="dfr")
                gj = mpool.tile([128, ncoarse], f32, tag="gj")
                nc.vector.tensor_scalar(t1[:, :], cP[:, :], 1.0 / WIN, None,
                                        op0=mybir.AluOpType.mult)
                nc.vector.tensor_scalar(t2[:, :], t1[:, :], MAGF, MAGF,
                                        op0=mybir.AluOpType.add,
                                        op1=mybir.AluOpType.subtract)
                nc.vector.tensor_tensor(out=dfr[:, :], in0=t1[:, :], in1=t2[:, :],
                                        op=mybir.AluOpType.subtract)
                # gj = 512*d + 512*(d<0) + wbase
                nc.vector.tensor_scalar(t2[:, :], dfr[:, :], 0.0, float(WIN),
                                        op0=mybir.AluOpType.is_lt,
                                        op1=mybir.AluOpType.mult)
                nc.vector.tensor_scalar(t1[:, :], dfr[:, :], float(WIN), None,
                                        op0=mybir.AluOpType.mult)
                nc.vector.tensor_tensor(out=gj[:, :], in0=t1[:, :], in1=t2[:, :],
                                        op=mybir.AluOpType.add)
                nc.vector.tensor_tensor(out=gj[:, :], in0=gj[:, :], in1=wbase[:, :],
                                        op=mybir.AluOpType.add)

                # mark top-16 coarse slots in-place (by packed value)
                m8a = mpool.tile([128, 8], f32, tag="m8a")
                m8b = mpool.tile([128, 8], f32, tag="m8b")
                zap = mpool.tile([128, ncoarse], f32, tag="zap")
                nc.vector.max(m8a[:, :], cP[:, :])
                nc.vector.match_replace(zap[:, :], m8a[:, :], cP[:, :], NEG)
                nc.vector.max(m8b[:, :], zap[:, :])
                nc.vector.match_replace(zap[:, :], m8b[:, :], zap[:, :], NEG)

                # compact: packed = 2^20 * is_marked + gj, top-16 of packed
                mask = mpool.tile([128, ncoarse], f32, tag="mask")
                nc.vector.tensor_scalar(
                    mask[:, :], zap[:, :], -1.0e38, MARK,
                    op0=mybir.AluOpType.is_le, op1=mybir.AluOpType.mult,
                )
                nc.vector.tensor_tensor(out=mask[:, :], in0=mask[:, :], in1=gj[:, :],
                                        op=mybir.AluOpType.add)
                p8a = mpool.tile([128, 8], f32, tag="p8a")
                p8b = mpool.tile([128, 8], f32, tag="p8b")
                nc.vector.max(p8a[:, :], mask[:, :])
                nc.vector.match_replace(mask[:, :], p8a[:, :], mask[:, :], NEG)
                nc.vector.max(p8b[:, :], mask[:, :])

                j16f = mpool.tile([128, 2 * 8], f32, tag="j16f")
                nc.vector.tensor_scalar(
                    j16f[:, 0:8], p8a[:, :], MARK, None,
                    op0=mybir.AluOpType.subtract,
                )
                nc.vector.tensor_scalar(
                    j16f[:, 8:16], p8b[:, :], MARK, None,
                    op0=mybir.AluOpType.subtract,
                )
                nc.vector.tensor_copy(jall[:, b * k:(b + 1) * k], j16f[:, :])

            def transpose_half(h):
                """Transpose jall cols [h*128,(h+1)*128) and replicate to jrep.

                The replication copies are split across both HWDGE queues
                (SP + Activation) -- serialized on one queue they gate the
                MLP gathers for ~600ns apiece."""
                nc.sync.dma_start_transpose(
                    jT[:, h * 128:(h + 1) * 128], jall[:, h * 128:(h + 1) * 128]
                )
                qi = 0
                for b in range(h * 8, (h + 1) * 8):
                    hb, p0 = divmod(b * k, 128)
                    src = jT[p0:p0 + k, hb * 128:hb * 128 + BLK]
                    for g in range(8):
                        eng = nc.sync if qi % 2 == 0 else nc.scalar
                        eng.dma_start(jrep[b][16 * g:16 * g + k, :], src)
                        qi += 1

            def mlp_chunk(ch):
                chunks_per_blk = BLK // rows_per_chunk
                b, sub = divmod(ch, chunks_per_blk)
                r0 = b * BLK + sub * rows_per_chunk
                idxs = jrep[b][:, sub * rows_per_chunk:(sub + 1) * rows_per_chunk]

                qsel = dpool.tile([128, CHUNK], f32, tag="qsel")
                nc.gpsimd.ap_gather(
                    qsel[:, :], qT[:, :], idxs,
                    channels=128, num_elems=n, d=1, num_idxs=CHUNK,
                )
                h1p = dpool.tile([128, CHUNK], f32, tag="h1p")
                pbc = (
                    pT[:, r0:r0 + rows_per_chunk]
                    .rearrange("p (r o) -> p r o", o=1)
                    .to_broadcast([c, rows_per_chunk, k])
                )
                nc.vector.tensor_tensor(
                    out=h1p[:, :],
                    in0=qsel[:, :].rearrange("p (r k) -> p r k", k=k),
                    in1=pbc,
                    op=mybir.AluOpType.add,
                )
                h1 = dpool.tile([128, CHUNK], f16, tag="h1")
                nc.scalar.activation(
                    h1[:, :], h1p[:, :], mybir.ActivationFunctionType.Relu,
                    bias=b1[:, :],
                )
                ps2 = ppB.tile([128, CHUNK], f32, tag="mm2")
                nc.tensor.matmul(ps2[:, :], w2[:, :], h1[:, :])
                h2 = dpool.tile([128, CHUNK], f32, tag="h2")
                nc.scalar.activation(
                    h2[:, :], ps2[:, :], mybir.ActivationFunctionType.Relu,
                    bias=b2s[:, :], scale=1.0 / k,
                )
                nc.vector.tensor_reduce(
                    out=outT[:, r0:r0 + rows_per_chunk],
                    in_=h2[:, :].rearrange("p (r k) -> p r k", k=k),
                    op=mybir.AluOpType.add,
                    axis=mybir.AxisListType.X,
                )

            # interleave: screen blocks 0-7, then their MLP overlaps blocks 8-15
            chunks_per_blk = BLK // rows_per_chunk
            for b in range(8):
                screen_block(b)
            transpose_half(0)
            for b in range(8, nblk):
                screen_block(b)
            transpose_half(1)
            for ch in range(nchunk):
                mlp_chunk(ch)

            nc.sync.dma_start(out_d[:, :], outT[:, :])

    nc.finalize()
    return nc


def host_prep(x, W1, b1, W2, b2, n=N, d=D, c=C, k=K, rpc=RPC, ncores=NCORES):
    x = np.ascontiguousarray(np.asarray(x, dtype=np.float32))
    W1 = np.asarray(W1, dtype=np.float32)
    b1 = np.asarray(b1, dtype=np.float32)
    W2 = np.asarray(W2, dtype=np.float32)
    b2 = np.asarray(b2, dtype=np.float32)

    sq = np.sum(x * x, axis=1, dtype=np.float32)
    nwin = n // WIN
    ncoarse = nwin * 8
    a = np.float32(np.sqrt(2.0 * CSCALE))

    # moving operand: rows 0:d = a*x^T; d,d+1 = (sq-SHIFT) hi/lo; d+2.. = magic
    s = (sq - np.float32(SHIFT)).astype(np.float32)
    sh = s.astype(np.float16)
    sl = (s.astype(np.float64) - sh.astype(np.float64)).astype(np.float16)
    xaug = np.zeros((KA, n), dtype=np.float16)
    xaug[:d] = (x.T * a).astype(np.float16)
    xaug[d] = sh
    xaug[d + 1] = sl
    xaug[d + 2:d + 6] = np.float16(MAGR)

    unmw = np.full((4, 128), -MAGW, dtype=np.float16)
    unmr = np.full((4, WIN), MAGR, dtype=np.float16)
    iow = np.ones((1, 128), dtype=np.float16)
    ior = np.arange(WIN, dtype=np.float16).reshape(1, WIN)

    w1b = (W1[d:] / a).astype(np.float16)
    w1d = ((W1[:d] - W1[d:]) / a).astype(np.float16)
    w2 = W2.astype(np.float16)
    b1c = b1.reshape(c, 1).astype(np.float32)
    b2s = (b2 / k).reshape(c, 1).astype(np.float32)
    wbase = np.repeat(
        (np.arange(nwin, dtype=np.float32) * WIN), 8
    )[None, :].repeat(128, axis=0).astype(np.float32)
    wbase = np.ascontiguousarray(wbase[:, :ncoarse])

    in_maps = []
    for cid in range(ncores):
        rows = x[cid * rpc:(cid + 1) * rpc]
        sqr = sq[cid * rpc:(cid + 1) * rpc]
        wloc = np.zeros((KA, rpc), dtype=np.float16)
        wloc[:d] = (rows.T * a).astype(np.float16)
        wloc[d] = np.float16(-CSCALE)
        wloc[d + 1] = np.float16(-CSCALE)
        wloc[d + 2:d + 6] = np.float16(MAGW)
        in_maps.append(
            dict(
                xaug=xaug, wloc=np.ascontiguousarray(wloc), unmw=unmw, unmr=unmr,
                iow=iow, ior=ior, w1b=w1b, w1d=w1d, w2=w2, b1c=b1c, b2s=b2s,
                wbase=wbase,
            )
        )
    return in_maps


_NC_CACHE = {}


def kernel(x, W1, b1, W2, b2):
    _ensure_path()
    from concourse.bass_utils import run_bass_kernel_spmd

    key = "full"
    if key not in _NC_CACHE:
        _NC_CACHE[key] = build_program()
    nc = _NC_CACHE[key]

    in_maps = host_prep(x, W1, b1, W2, b2)
    res = run_bass_kernel_spmd(
        nc, in_maps, core_ids=list(range(NCORES)),
        trace=bool(int(os.environ.get("DGCNN_TRACE", "0"))),
    )
    out = np.empty((N, C), dtype=np.float32)
    for cid in range(NCORES):
        out[cid * RPC:(cid + 1) * RPC] = res.results[cid]["outT"].T
    if getattr(res, "exec_time_ns", None):
        kernel.last_exec_time_ns = res.exec_time_ns
    return out


kernel.last_exec_time_ns = None
